# revision 16
# baseline (speedup 1.0000x reference)
"""Trainium2 Bass kernel for nn_AttentionModule (sparse_attention).

Computation (per batch b):
  qe = LN(MLP(q[b]))            (16,)
  ke = LN(MLP(k[b,:,0,:].T))    (4, 16)
  logits = qe @ ke.T * DIM^-0.5 - modality_dropout*1e5
  attn = softmax(logits / 10)   (4,)
  out[b, vc, p] = sum_c attn[c] * v[b, vc, p, c]
  attn_map = broadcast(attn)

Sharding: pure data parallel over batch across 8 NeuronCores (8 batches each).
MLP/LN params replicated. The heavy part (streaming 4 MB of v per batch) is
memory bound: per core 32 MB in + 8 MB out ~= 111 us at ~360 GB/s.

Device layout per batch: v[b] viewed flat as (128, 7840) f32 (partition-
contiguous DMA); channel c lives at free-dim stride 4. Weighted sum =
4 ScalarE multiplies (per-partition scale = attn weight broadcast via a
tiny TensorE outer-product) + 3 VectorE adds; result (128, 1960) DMAs out
contiguously. attn_map is a pure broadcast of the (64,4) attn tensor, so it
is materialized host-side from the device-computed attn.
"""

import numpy as np
from contextlib import ExitStack

import concourse.bass as bass
import concourse.tile as tile
from concourse import mybir
from concourse.bass_utils import run_bass_kernel_spmd

N_CORES = 8
B = 64
BPC = B // N_CORES  # batches per core
V_CH = 5
P = 50176
C = 4
DIM = 10
IMG = 224
ROWS = 128
FW = V_CH * P * C // ROWS   # 7840 f32 per partition per batch (with channels)
FO = V_CH * P // ROWS       # 1960 outputs per partition per batch
SCALE_T = float(DIM ** -0.5) / 10.0   # attn scale folded with temperature
MD_SCALE = 1e5 / 10.0
LN_EPS = 1e-5
F32 = mybir.dt.float32
AL = mybir.AluOpType
AF = mybir.ActivationFunctionType
AX = mybir.AxisListType


def _cap_sync_waits(nc, max_waits=1):
    """walrus in this container rejects >1 sync-wait per instruction ("Too
    many sync wait commands"). Spill excess waits onto same-engine nops
    inserted immediately before the instruction (engine streams execute bb
    instructions in order, so the waits still complete first)."""

    def _pop_by_name(name):
        for f2 in nc.m.functions:
            for b2 in f2.blocks:
                lst = b2.instructions
                for j in range(len(lst) - 1, -1, -1):
                    if lst[j].name == name:
                        return lst.pop(j)
        raise RuntimeError(f"spill nop {name} not found")

    for fn in nc.m.functions:
        for blk in fn.blocks:
            insts = blk.instructions
            i = 0
            while i < len(insts):
                ins = insts[i]
                si = ins.sync_info
                waits = list(si.on_wait) if si is not None and si.on_wait else []
                keep = 0 if type(ins).__name__ == "InstDrain" else max_waits
                if len(waits) > keep:
                    upds = (
                        list(si.on_update)
                        if si is not None and si.on_update
                        else []
                    )
                    spill = waits[keep:]
                    ins.sync_info = mybir.SyncInfo(
                        on_wait=waits[:keep], on_update=upds
                    )
                    for w in spill:
                        bi = nc.engines[ins.engine].nop(
                            nofuse=True, hint="wait_spill"
                        )
                        nop_inst = _pop_by_name(bi.ins.name)
                        nop_inst.sync_info = mybir.SyncInfo(
                            on_wait=[w], on_update=[]
                        )
                        insts.insert(i, nop_inst)
                        i += 1
                i += 1


def _layer_norm(nc, pool, x, g, beta, np_, name, eps_tile):
    """LayerNorm over the free dim (16) of x (np_, 16). Returns SBUF tile."""
    stats = pool.tile([np_, nc.vector.BN_STATS_DIM], F32, tag=name + "_st")
    nc.vector.bn_stats(out=stats, in_=x)
    mv = pool.tile([np_, nc.vector.BN_AGGR_DIM], F32, tag=name + "_mv")
    nc.vector.bn_aggr(out=mv, in_=stats)
    xc = pool.tile([np_, 16], F32, tag=name + "_xc")
    nc.vector.tensor_scalar(
        out=xc, in0=x, scalar1=mv[:, 0:1], scalar2=None, op0=AL.subtract
    )
    std = pool.tile([np_, 1], F32, tag=name + "_sd")
    nc.scalar.activation(
        out=std, in_=mv[:, 1:2], func=AF.Sqrt, bias=eps_tile[0:np_]
    )
    rstd = pool.tile([np_, 1], F32, tag=name + "_rs")
    nc.vector.reciprocal(out=rstd, in_=std)
    y = pool.tile([np_, 16], F32, tag=name + "_y")
    nc.vector.tensor_scalar(
        out=y, in0=xc, scalar1=rstd, scalar2=None, op0=AL.mult
    )
    nc.vector.tensor_tensor(out=y, in0=y, in1=g, op=AL.mult)
    nc.vector.tensor_tensor(out=y, in0=y, in1=beta, op=AL.add)
    return y


PARAM_LAYOUT = [
    # (name, rows, cols) packed side by side into one (128, PARAM_K) array
    ("xq", DIM, BPC),
    ("xk", DIM, C * BPC),
    ("md", BPC, C),
    ("qw1", DIM, 128),
    ("qw2", 128, 16),
    ("kw1", DIM, 128),
    ("kw2", 128, 16),
    ("qb1", 128, 1),
    ("kb1", 128, 1),
    ("qb2", 32, 16),
    ("qg", 32, 16),
    ("qbe", 32, 16),
    ("kb2", 32, 16),
    ("kg", 32, 16),
    ("kbe", 32, 16),
    ("eye", 32, 32),
    ("mask", BPC, C * BPC),
    ("mask4", C * BPC, C),
    ("ones1", 1, 128),
    ("eps", 32, 1),
]
PARAM_OFF = {}
_off = 0
for _n, _r, _c in PARAM_LAYOUT:
    PARAM_OFF[_n] = _off
    _off += _c
PARAM_K = _off


def _build():
    nc = bass.Bass("TRN2", target_bir_lowering=False, debug=False)

    v_d = nc.dram_tensor("v", [BPC * ROWS, FW], F32, kind="ExternalInput")
    params_d = nc.dram_tensor("params", [128, PARAM_K], F32, kind="ExternalInput")

    out_d = nc.dram_tensor("out", [BPC * ROWS, FO], F32, kind="ExternalOutput")
    attn_d = nc.dram_tensor("attn", [BPC, C], F32, kind="ExternalOutput")

    with tile.TileContext(nc) as tc, ExitStack() as ctx:
        singles = ctx.enter_context(tc.tile_pool(name="singles", bufs=1))
        small = ctx.enter_context(tc.tile_pool(name="small", bufs=1))
        psum = ctx.enter_context(tc.tile_pool(name="psum", bufs=1, space="PSUM"))
        vpool = ctx.enter_context(tc.tile_pool(name="vpool", bufs=3))
        tpool = ctx.enter_context(tc.tile_pool(name="tpool", bufs=2))

        params = singles.tile([128, PARAM_K], F32, tag="params")
        nc.sync.dma_start(out=params, in_=params_d.ap())

        def pslice(name, rows, cols):
            off = PARAM_OFF[name]
            return params[0:rows, off : off + cols]

        views = {n: pslice(n, r, c) for n, r, c in PARAM_LAYOUT}
        xq = views["xq"]
        xk = views["xk"]
        mdt = views["md"]
        qw1 = views["qw1"]
        qw2 = views["qw2"]
        kw1 = views["kw1"]
        kw2 = views["kw2"]
        qb1 = views["qb1"]
        kb1 = views["kb1"]
        qb2 = views["qb2"]
        qg = views["qg"]
        qbe = views["qbe"]
        kb2 = views["kb2"]
        kg = views["kg"]
        kbe = views["kbe"]
        eye = views["eye"]
        mask = views["mask"]
        mask4 = views["mask4"]
        ones1 = views["ones1"]
        eps_t = views["eps"]

        # ---- q path MLP + LN -> qe (BPC, 16)
        h1q_ps = psum.tile([128, BPC], F32, tag="psA")
        nc.tensor.matmul(out=h1q_ps, lhsT=qw1, rhs=xq)
        # LeakyReLU(0.1) done manually: the ACT Lrelu table bakes alpha=0.01.
        h1q = small.tile([128, BPC], F32, tag="h1q_sb")
        h1qt = small.tile([128, BPC], F32, tag="h1q_t")
        nc.vector.tensor_scalar(
            out=h1q, in0=h1q_ps, scalar1=qb1, scalar2=None, op0=AL.add
        )
        nc.vector.tensor_scalar(
            out=h1qt, in0=h1q, scalar1=0.1, scalar2=None, op0=AL.mult
        )
        nc.vector.tensor_tensor(out=h1q, in0=h1q, in1=h1qt, op=AL.max)
        h2q_ps = psum.tile([BPC, 16], F32, tag="psB")
        nc.tensor.matmul(out=h2q_ps, lhsT=h1q, rhs=qw2)
        hq = small.tile([BPC, 16], F32, tag="hq")
        nc.vector.tensor_tensor(out=hq, in0=h2q_ps, in1=qb2[0:BPC], op=AL.add)
        qe = _layer_norm(nc, small, hq, qg[0:BPC], qbe[0:BPC], BPC, "qln", eps_t)

        # ---- k path MLP + LN -> ke (C*BPC, 16), row j = 4*b + c
        h1k_ps = psum.tile([128, C * BPC], F32, tag="psA")
        nc.tensor.matmul(out=h1k_ps, lhsT=kw1, rhs=xk)
        h1k = small.tile([128, C * BPC], F32, tag="h1k_sb")
        h1kt = small.tile([128, C * BPC], F32, tag="h1k_t")
        nc.vector.tensor_scalar(
            out=h1k, in0=h1k_ps, scalar1=kb1, scalar2=None, op0=AL.add
        )
        nc.vector.tensor_scalar(
            out=h1kt, in0=h1k, scalar1=0.1, scalar2=None, op0=AL.mult
        )
        nc.vector.tensor_tensor(out=h1k, in0=h1k, in1=h1kt, op=AL.max)
        h2k_ps = psum.tile([C * BPC, 16], F32, tag="psB")
        nc.tensor.matmul(out=h2k_ps, lhsT=h1k, rhs=kw2)
        hk = small.tile([C * BPC, 16], F32, tag="hk")
        nc.vector.tensor_tensor(out=hk, in0=h2k_ps, in1=kb2, op=AL.add)
        ke = _layer_norm(nc, small, hk, kg, kbe, C * BPC, "kln", eps_t)

        # ---- logits: L2[b', 4b+c] = qe[b'] . ke[4b+c]; keep diagonal b'=b
        qeT_ps = psum.tile([16, BPC], F32, tag="psC")
        nc.tensor.transpose(out=qeT_ps, in_=qe, identity=eye[0:BPC, 0:BPC])
        qeT = small.tile([16, BPC], F32, tag="qeT_sb")
        nc.vector.tensor_copy(out=qeT, in_=qeT_ps)
        keT_ps = psum.tile([16, C * BPC], F32, tag="psD")
        nc.tensor.transpose(out=keT_ps, in_=ke, identity=eye)
        keT = small.tile([16, C * BPC], F32, tag="keT_sb")
        nc.vector.tensor_copy(out=keT, in_=keT_ps)
        l2_ps = psum.tile([BPC, C * BPC], F32, tag="psB")
        nc.tensor.matmul(out=l2_ps, lhsT=qeT, rhs=keT)
        zm = small.tile([BPC, C * BPC], F32, tag="zm")
        nc.vector.tensor_tensor(out=zm, in0=l2_ps, in1=mask, op=AL.mult)
        z = small.tile([BPC, C], F32, tag="z")
        nc.vector.tensor_reduce(
            out=z,
            in_=zm.rearrange("p (b2 c) -> p c b2", c=C),
            axis=AX.X,
            op=AL.add,
        )

        # ---- masked softmax over the 4 modalities
        nc.vector.tensor_scalar(
            out=z, in0=z, scalar1=SCALE_T, scalar2=None, op0=AL.mult
        )
        mds = small.tile([BPC, C], F32, tag="mds")
        nc.vector.tensor_scalar(
            out=mds, in0=mdt, scalar1=MD_SCALE, scalar2=None, op0=AL.mult
        )
        nc.vector.tensor_tensor(out=z, in0=z, in1=mds, op=AL.subtract)
        rmax = small.tile([BPC, 1], F32, tag="rmax")
        nc.vector.tensor_reduce(out=rmax, in_=z, axis=AX.X, op=AL.max)
        nmax = small.tile([BPC, 1], F32, tag="nmax")
        nc.vector.tensor_scalar(
            out=nmax, in0=rmax, scalar1=-1.0, scalar2=None, op0=AL.mult
        )
        e = small.tile([BPC, C], F32, tag="e")
        nc.scalar.activation(out=e, in_=z, func=AF.Exp, bias=nmax, scale=1.0)
        ssum = small.tile([BPC, 1], F32, tag="ssum")
        nc.vector.tensor_reduce(out=ssum, in_=e, axis=AX.X, op=AL.add)
        rs = small.tile([BPC, 1], F32, tag="rs")
        nc.vector.reciprocal(out=rs, in_=ssum)
        attn_sb = small.tile([BPC, C], F32, tag="attn_sb")
        nc.vector.tensor_scalar(
            out=attn_sb, in0=e, scalar1=rs, scalar2=None, op0=AL.mult
        )
        nc.scalar.dma_start(out=attn_d.ap(), in_=attn_sb)

        # ---- broadcast attn to all 128 partitions: wall[p, 4b+c] = attn[b, c]
        # 1) scatter rows: o32[4b+c', c] = attn[b, c] (mask == repeat(eye(8),4))
        o32_ps = psum.tile([C * BPC, C], F32, tag="psB")
        nc.tensor.matmul(out=o32_ps, lhsT=mask, rhs=attn_sb)
        # 2) keep diagonal c' == j%4 and reduce -> attn32[4b+c] = attn[b, c]
        o32 = small.tile([C * BPC, C], F32, tag="o32_sb")
        nc.vector.tensor_tensor(out=o32, in0=o32_ps, in1=mask4, op=AL.mult)
        attn32 = small.tile([C * BPC, 1], F32, tag="attn32")
        nc.vector.tensor_reduce(out=attn32, in_=o32, axis=AX.X, op=AL.add)
        # 3) flatten to one partition, then outer-product with a ones column
        arow_ps = psum.tile([1, C * BPC], F32, tag="psC")
        nc.tensor.transpose(out=arow_ps, in_=attn32, identity=eye)
        arow = small.tile([1, C * BPC], F32, tag="arow_sb")
        nc.vector.tensor_copy(out=arow, in_=arow_ps)
        wps = psum.tile([128, C * BPC], F32, tag="psD")
        nc.tensor.matmul(out=wps, lhsT=ones1, rhs=arow)
        wall = singles.tile([128, C * BPC], F32, tag="wall_sb")
        nc.vector.tensor_copy(out=wall, in_=wps)

        # ---- big weighted sum over v, one batch at a time
        v_ap = v_d.ap()
        o_ap = out_d.ap()
        for b in range(BPC):
            T = vpool.tile([ROWS, FW], F32, tag="T")
            nc.sync.dma_start(out=T, in_=v_ap[b * ROWS : (b + 1) * ROWS, :])
            Tv = T.rearrange("p (f c) -> p c f", c=C)  # channel slice = stride 4
            tts = []
            for cc in range(C):
                tcc = tpool.tile([ROWS, FO], F32, tag=f"t{cc}")
                w_ap = wall[:, C * b + cc : C * b + cc + 1]
                ch = Tv[:, cc : cc + 1, :].squeeze(1)
                if cc % 2 == 0:
                    nc.scalar.mul(out=tcc, in_=ch, mul=w_ap)
                else:
                    nc.vector.tensor_scalar(
                        out=tcc, in0=ch, scalar1=w_ap, scalar2=None, op0=AL.mult
                    )
                tts.append(tcc)
            nc.vector.tensor_tensor(out=tts[0], in0=tts[0], in1=tts[1], op=AL.add)
            nc.vector.tensor_tensor(out=tts[2], in0=tts[2], in1=tts[3], op=AL.add)
            nc.vector.tensor_tensor(out=tts[0], in0=tts[0], in1=tts[2], op=AL.add)
            # out-DMAs ride the ACT HWDGE ring so they don't serialize
            # against the next batch's 4MB input on the SP ring (FIFO).
            nc.scalar.dma_start(out=o_ap[b * ROWS : (b + 1) * ROWS, :], in_=tts[0])

    _cap_sync_waits(nc)
    return nc


_CACHE = {}


def _get_nc():
    if "nc" not in _CACHE:
        _CACHE["nc"] = _build()
    return _CACHE["nc"]


def _pack_params(blocks):
    """Pack named (rows, cols) blocks into one (128, PARAM_K) f32 array."""
    A = np.zeros((128, PARAM_K), np.float32)
    for name, rows, cols in PARAM_LAYOUT:
        blk = blocks[name]
        assert blk.shape == (rows, cols), (name, blk.shape, rows, cols)
        off = PARAM_OFF[name]
        A[0:rows, off : off + cols] = blk
    return A


def _make_in_maps(q, k, v, modality_dropout,
                  q_w1, q_b1, q_w2, q_b2, q_g, q_beta,
                  k_w1, k_b1, k_w2, k_b2, k_g, k_beta):
    f = np.float32
    q = np.asarray(q, dtype=f)
    k = np.asarray(k, dtype=f)
    v = np.asarray(v, dtype=f)
    md = np.asarray(modality_dropout, dtype=f)
    const_blocks = {
        "qw1": np.asarray(q_w1, f).T,
        "qw2": np.asarray(q_w2, f).T,
        "kw1": np.asarray(k_w1, f).T,
        "kw2": np.asarray(k_w2, f).T,
        "qb1": np.asarray(q_b1, f).reshape(128, 1),
        "kb1": np.asarray(k_b1, f).reshape(128, 1),
        "qb2": np.tile(np.asarray(q_b2, f), (32, 1)),
        "qg": np.tile(np.asarray(q_g, f), (32, 1)),
        "qbe": np.tile(np.asarray(q_beta, f), (32, 1)),
        "kb2": np.tile(np.asarray(k_b2, f), (32, 1)),
        "kg": np.tile(np.asarray(k_g, f), (32, 1)),
        "kbe": np.tile(np.asarray(k_beta, f), (32, 1)),
        "eye": np.eye(32, dtype=f),
        "mask": np.repeat(np.eye(BPC, dtype=f), C, axis=1),
        "mask4": np.tile(np.eye(C, dtype=f), (BPC, 1)),
        "ones1": np.ones((1, 128), dtype=f),
        "eps": np.full((32, 1), LN_EPS, dtype=f),
    }
    in_maps = []
    for m in range(N_CORES):
        sl = slice(m * BPC, (m + 1) * BPC)
        blocks = dict(const_blocks)
        blocks["xq"] = q[sl, :, 0].T
        blocks["xk"] = np.transpose(k[sl, :, 0, :], (1, 0, 2)).reshape(
            DIM, C * BPC
        )
        blocks["md"] = md[sl]
        in_maps.append({
            "v": np.ascontiguousarray(v[sl]).reshape(BPC * ROWS, FW),
            "params": _pack_params(blocks),
        })
    return in_maps


def _run(in_maps, **kwargs):
    nc = _get_nc()
    return run_bass_kernel_spmd(nc, in_maps, core_ids=list(range(N_CORES)), **kwargs)


def _gather(res):
    out_full = np.empty((B, V_CH, IMG, IMG), np.float32)
    attn_full = np.empty((B, C), np.float32)
    for m in range(N_CORES):
        r = res.results[m]
        out_full[m * BPC : (m + 1) * BPC] = r["out"].reshape(BPC, V_CH, IMG, IMG)
        attn_full[m * BPC : (m + 1) * BPC] = r["attn"]
    attn_map = np.ascontiguousarray(
        np.broadcast_to(attn_full[:, :, None, None], (B, C, IMG, IMG))
    )
    return out_full, attn_map


def kernel(**inputs):
    in_maps = _make_in_maps(**inputs)
    res = _run(in_maps)
    return _gather(res)


def kernel_profiled(**inputs):
    """Like kernel(), but also returns BassKernelResults with trace info."""
    in_maps = _make_in_maps(**inputs)
    res = _run(in_maps, trace=True)
    return _gather(res), res


# revision 17
# speedup vs baseline: 1.0416x; 1.0416x over previous
"""Trainium2 Bass kernel for nn_AttentionModule (sparse_attention).

Computation (per batch b):
  qe = LN(MLP(q[b]))            (16,)
  ke = LN(MLP(k[b,:,0,:].T))    (4, 16)
  logits = qe @ ke.T * DIM^-0.5 - modality_dropout*1e5
  attn = softmax(logits / 10)   (4,)
  out[b, vc, p] = sum_c attn[c] * v[b, vc, p, c]
  attn_map = broadcast(attn)

Sharding: pure data parallel over batch across 8 NeuronCores (8 batches each).
MLP/LN params replicated. The heavy part (streaming 4 MB of v per batch) is
memory bound: per core 32 MB in + 8 MB out ~= 111 us at ~360 GB/s.

Device layout per batch: v[b] viewed flat as (128, 7840) f32 (partition-
contiguous DMA); channel c lives at free-dim stride 4. Weighted sum =
4 ScalarE multiplies (per-partition scale = attn weight broadcast via a
tiny TensorE outer-product) + 3 VectorE adds; result (128, 1960) DMAs out
contiguously. attn_map is a pure broadcast of the (64,4) attn tensor, so it
is materialized host-side from the device-computed attn.
"""

import numpy as np
from contextlib import ExitStack

import concourse.bass as bass
import concourse.tile as tile
from concourse import mybir
from concourse.bass_utils import run_bass_kernel_spmd

N_CORES = 8
B = 64
BPC = B // N_CORES  # batches per core
V_CH = 5
P = 50176
C = 4
DIM = 10
IMG = 224
ROWS = 128
FW = V_CH * P * C // ROWS   # 7840 f32 per partition per batch (with channels)
FO = V_CH * P // ROWS       # 1960 outputs per partition per batch
SCALE_T = float(DIM ** -0.5) / 10.0   # attn scale folded with temperature
MD_SCALE = 1e5 / 10.0
LN_EPS = 1e-5
F32 = mybir.dt.float32
AL = mybir.AluOpType
AF = mybir.ActivationFunctionType
AX = mybir.AxisListType


def _cap_sync_waits(nc, max_waits=1):
    """walrus in this container rejects >1 sync-wait per instruction ("Too
    many sync wait commands"). Spill excess waits onto same-engine nops
    inserted immediately before the instruction (engine streams execute bb
    instructions in order, so the waits still complete first)."""

    def _pop_by_name(name):
        for f2 in nc.m.functions:
            for b2 in f2.blocks:
                lst = b2.instructions
                for j in range(len(lst) - 1, -1, -1):
                    if lst[j].name == name:
                        return lst.pop(j)
        raise RuntimeError(f"spill nop {name} not found")

    for fn in nc.m.functions:
        for blk in fn.blocks:
            insts = blk.instructions
            i = 0
            while i < len(insts):
                ins = insts[i]
                si = ins.sync_info
                waits = list(si.on_wait) if si is not None and si.on_wait else []
                keep = 0 if type(ins).__name__ == "InstDrain" else max_waits
                if len(waits) > keep:
                    upds = (
                        list(si.on_update)
                        if si is not None and si.on_update
                        else []
                    )
                    spill = waits[keep:]
                    ins.sync_info = mybir.SyncInfo(
                        on_wait=waits[:keep], on_update=upds
                    )
                    for w in spill:
                        bi = nc.engines[ins.engine].nop(
                            nofuse=True, hint="wait_spill"
                        )
                        nop_inst = _pop_by_name(bi.ins.name)
                        nop_inst.sync_info = mybir.SyncInfo(
                            on_wait=[w], on_update=[]
                        )
                        insts.insert(i, nop_inst)
                        i += 1
                i += 1


def _layer_norm(nc, pool, x, g, beta, np_, name, eps_tile):
    """LayerNorm over the free dim (16) of x (np_, 16). Returns SBUF tile."""
    stats = pool.tile([np_, nc.vector.BN_STATS_DIM], F32, tag=name + "_st")
    nc.vector.bn_stats(out=stats, in_=x)
    mv = pool.tile([np_, nc.vector.BN_AGGR_DIM], F32, tag=name + "_mv")
    nc.vector.bn_aggr(out=mv, in_=stats)
    xc = pool.tile([np_, 16], F32, tag=name + "_xc")
    nc.vector.tensor_scalar(
        out=xc, in0=x, scalar1=mv[:, 0:1], scalar2=None, op0=AL.subtract
    )
    std = pool.tile([np_, 1], F32, tag=name + "_sd")
    nc.scalar.activation(
        out=std, in_=mv[:, 1:2], func=AF.Sqrt, bias=eps_tile[0:np_]
    )
    rstd = pool.tile([np_, 1], F32, tag=name + "_rs")
    nc.vector.reciprocal(out=rstd, in_=std)
    y = pool.tile([np_, 16], F32, tag=name + "_y")
    nc.vector.tensor_scalar(
        out=y, in0=xc, scalar1=rstd, scalar2=None, op0=AL.mult
    )
    nc.vector.tensor_tensor(out=y, in0=y, in1=g, op=AL.mult)
    nc.vector.tensor_tensor(out=y, in0=y, in1=beta, op=AL.add)
    return y


PARAM_LAYOUT = [
    # (name, rows, cols) packed side by side into one (128, PARAM_K) array
    ("xq", DIM, BPC),
    ("xk", DIM, C * BPC),
    ("md", BPC, C),
    ("qw1", DIM, 128),
    ("qw2", 128, 16),
    ("kw1", DIM, 128),
    ("kw2", 128, 16),
    ("qb1", 128, 1),
    ("kb1", 128, 1),
    ("qb2", 32, 16),
    ("qg", 32, 16),
    ("qbe", 32, 16),
    ("kb2", 32, 16),
    ("kg", 32, 16),
    ("kbe", 32, 16),
    ("eye", 32, 32),
    ("mask", BPC, C * BPC),
    ("mask4", C * BPC, C),
    ("ones1", 1, 128),
    ("eps", 32, 1),
]
PARAM_OFF = {}
_off = 0
for _n, _r, _c in PARAM_LAYOUT:
    PARAM_OFF[_n] = _off
    _off += _c
PARAM_K = _off


def _build():
    nc = bass.Bass("TRN2", target_bir_lowering=False, debug=False)

    v_d = nc.dram_tensor("v", [BPC * ROWS, FW], F32, kind="ExternalInput")
    params_d = nc.dram_tensor("params", [128, PARAM_K], F32, kind="ExternalInput")

    out_d = nc.dram_tensor("out", [BPC * ROWS, FO], F32, kind="ExternalOutput")
    attn_d = nc.dram_tensor("attn", [BPC, C], F32, kind="ExternalOutput")

    with tile.TileContext(nc) as tc, ExitStack() as ctx:
        singles = ctx.enter_context(tc.tile_pool(name="singles", bufs=1))
        small = ctx.enter_context(tc.tile_pool(name="small", bufs=1))
        psum = ctx.enter_context(tc.tile_pool(name="psum", bufs=1, space="PSUM"))
        vpool = ctx.enter_context(tc.tile_pool(name="vpool", bufs=3))
        tpool = ctx.enter_context(tc.tile_pool(name="tpool", bufs=3))

        params = singles.tile([128, PARAM_K], F32, tag="params")
        nc.sync.dma_start(out=params, in_=params_d.ap())

        def pslice(name, rows, cols):
            off = PARAM_OFF[name]
            return params[0:rows, off : off + cols]

        views = {n: pslice(n, r, c) for n, r, c in PARAM_LAYOUT}
        xq = views["xq"]
        xk = views["xk"]
        mdt = views["md"]
        qw1 = views["qw1"]
        qw2 = views["qw2"]
        kw1 = views["kw1"]
        kw2 = views["kw2"]
        qb1 = views["qb1"]
        kb1 = views["kb1"]
        qb2 = views["qb2"]
        qg = views["qg"]
        qbe = views["qbe"]
        kb2 = views["kb2"]
        kg = views["kg"]
        kbe = views["kbe"]
        eye = views["eye"]
        mask = views["mask"]
        mask4 = views["mask4"]
        ones1 = views["ones1"]
        eps_t = views["eps"]

        # ---- q path MLP + LN -> qe (BPC, 16)
        h1q_ps = psum.tile([128, BPC], F32, tag="psA")
        nc.tensor.matmul(out=h1q_ps, lhsT=qw1, rhs=xq)
        # LeakyReLU(0.1) done manually: the ACT Lrelu table bakes alpha=0.01.
        h1q = small.tile([128, BPC], F32, tag="h1q_sb")
        h1qt = small.tile([128, BPC], F32, tag="h1q_t")
        nc.vector.tensor_scalar(
            out=h1q, in0=h1q_ps, scalar1=qb1, scalar2=None, op0=AL.add
        )
        nc.vector.tensor_scalar(
            out=h1qt, in0=h1q, scalar1=0.1, scalar2=None, op0=AL.mult
        )
        nc.vector.tensor_tensor(out=h1q, in0=h1q, in1=h1qt, op=AL.max)
        h2q_ps = psum.tile([BPC, 16], F32, tag="psB")
        nc.tensor.matmul(out=h2q_ps, lhsT=h1q, rhs=qw2)
        hq = small.tile([BPC, 16], F32, tag="hq")
        nc.vector.tensor_tensor(out=hq, in0=h2q_ps, in1=qb2[0:BPC], op=AL.add)
        qe = _layer_norm(nc, small, hq, qg[0:BPC], qbe[0:BPC], BPC, "qln", eps_t)

        # ---- k path MLP + LN -> ke (C*BPC, 16), row j = 4*b + c
        h1k_ps = psum.tile([128, C * BPC], F32, tag="psA")
        nc.tensor.matmul(out=h1k_ps, lhsT=kw1, rhs=xk)
        h1k = small.tile([128, C * BPC], F32, tag="h1k_sb")
        h1kt = small.tile([128, C * BPC], F32, tag="h1k_t")
        nc.vector.tensor_scalar(
            out=h1k, in0=h1k_ps, scalar1=kb1, scalar2=None, op0=AL.add
        )
        nc.vector.tensor_scalar(
            out=h1kt, in0=h1k, scalar1=0.1, scalar2=None, op0=AL.mult
        )
        nc.vector.tensor_tensor(out=h1k, in0=h1k, in1=h1kt, op=AL.max)
        h2k_ps = psum.tile([C * BPC, 16], F32, tag="psB")
        nc.tensor.matmul(out=h2k_ps, lhsT=h1k, rhs=kw2)
        hk = small.tile([C * BPC, 16], F32, tag="hk")
        nc.vector.tensor_tensor(out=hk, in0=h2k_ps, in1=kb2, op=AL.add)
        ke = _layer_norm(nc, small, hk, kg, kbe, C * BPC, "kln", eps_t)

        # ---- logits: L2[b', 4b+c] = qe[b'] . ke[4b+c]; keep diagonal b'=b
        qeT_ps = psum.tile([16, BPC], F32, tag="psC")
        nc.tensor.transpose(out=qeT_ps, in_=qe, identity=eye[0:BPC, 0:BPC])
        qeT = small.tile([16, BPC], F32, tag="qeT_sb")
        nc.vector.tensor_copy(out=qeT, in_=qeT_ps)
        keT_ps = psum.tile([16, C * BPC], F32, tag="psD")
        nc.tensor.transpose(out=keT_ps, in_=ke, identity=eye)
        keT = small.tile([16, C * BPC], F32, tag="keT_sb")
        nc.vector.tensor_copy(out=keT, in_=keT_ps)
        l2_ps = psum.tile([BPC, C * BPC], F32, tag="psB")
        nc.tensor.matmul(out=l2_ps, lhsT=qeT, rhs=keT)
        zm = small.tile([BPC, C * BPC], F32, tag="zm")
        nc.vector.tensor_tensor(out=zm, in0=l2_ps, in1=mask, op=AL.mult)
        z = small.tile([BPC, C], F32, tag="z")
        nc.vector.tensor_reduce(
            out=z,
            in_=zm.rearrange("p (b2 c) -> p c b2", c=C),
            axis=AX.X,
            op=AL.add,
        )

        # ---- masked softmax over the 4 modalities
        nc.vector.tensor_scalar(
            out=z, in0=z, scalar1=SCALE_T, scalar2=None, op0=AL.mult
        )
        mds = small.tile([BPC, C], F32, tag="mds")
        nc.vector.tensor_scalar(
            out=mds, in0=mdt, scalar1=MD_SCALE, scalar2=None, op0=AL.mult
        )
        nc.vector.tensor_tensor(out=z, in0=z, in1=mds, op=AL.subtract)
        rmax = small.tile([BPC, 1], F32, tag="rmax")
        nc.vector.tensor_reduce(out=rmax, in_=z, axis=AX.X, op=AL.max)
        nmax = small.tile([BPC, 1], F32, tag="nmax")
        nc.vector.tensor_scalar(
            out=nmax, in0=rmax, scalar1=-1.0, scalar2=None, op0=AL.mult
        )
        e = small.tile([BPC, C], F32, tag="e")
        nc.scalar.activation(out=e, in_=z, func=AF.Exp, bias=nmax, scale=1.0)
        ssum = small.tile([BPC, 1], F32, tag="ssum")
        nc.vector.tensor_reduce(out=ssum, in_=e, axis=AX.X, op=AL.add)
        rs = small.tile([BPC, 1], F32, tag="rs")
        nc.vector.reciprocal(out=rs, in_=ssum)
        attn_sb = small.tile([BPC, C], F32, tag="attn_sb")
        nc.vector.tensor_scalar(
            out=attn_sb, in0=e, scalar1=rs, scalar2=None, op0=AL.mult
        )
        nc.sync.dma_start(out=attn_d.ap(), in_=attn_sb)

        # ---- broadcast attn to all 128 partitions: wall[p, 4b+c] = attn[b, c]
        # 1) scatter rows: o32[4b+c', c] = attn[b, c] (mask == repeat(eye(8),4))
        o32_ps = psum.tile([C * BPC, C], F32, tag="psB")
        nc.tensor.matmul(out=o32_ps, lhsT=mask, rhs=attn_sb)
        # 2) keep diagonal c' == j%4 and reduce -> attn32[4b+c] = attn[b, c]
        o32 = small.tile([C * BPC, C], F32, tag="o32_sb")
        nc.vector.tensor_tensor(out=o32, in0=o32_ps, in1=mask4, op=AL.mult)
        attn32 = small.tile([C * BPC, 1], F32, tag="attn32")
        nc.vector.tensor_reduce(out=attn32, in_=o32, axis=AX.X, op=AL.add)
        # 3) flatten to one partition, then outer-product with a ones column
        arow_ps = psum.tile([1, C * BPC], F32, tag="psC")
        nc.tensor.transpose(out=arow_ps, in_=attn32, identity=eye)
        arow = small.tile([1, C * BPC], F32, tag="arow_sb")
        nc.vector.tensor_copy(out=arow, in_=arow_ps)
        wps = psum.tile([128, C * BPC], F32, tag="psD")
        nc.tensor.matmul(out=wps, lhsT=ones1, rhs=arow)
        wall = singles.tile([128, C * BPC], F32, tag="wall_sb")
        nc.vector.tensor_copy(out=wall, in_=wps)

        # ---- big weighted sum over v, one batch at a time
        v_ap = v_d.ap()
        o_ap = out_d.ap()
        for b in range(BPC):
            T = vpool.tile([ROWS, FW], F32, tag="T")
            nc.sync.dma_start(out=T, in_=v_ap[b * ROWS : (b + 1) * ROWS, :])
            Tv = T.rearrange("p (f c) -> p c f", c=C)  # channel slice = stride 4
            tts = []
            for cc in range(C):
                tcc = tpool.tile([ROWS, FO], F32, tag=f"t{cc}")
                w_ap = wall[:, C * b + cc : C * b + cc + 1]
                ch = Tv[:, cc : cc + 1, :].squeeze(1)
                if cc % 2 == 0:
                    nc.scalar.mul(out=tcc, in_=ch, mul=w_ap)
                else:
                    nc.vector.tensor_scalar(
                        out=tcc, in0=ch, scalar1=w_ap, scalar2=None, op0=AL.mult
                    )
                tts.append(tcc)
            nc.vector.tensor_tensor(out=tts[0], in0=tts[0], in1=tts[1], op=AL.add)
            nc.vector.tensor_tensor(out=tts[2], in0=tts[2], in1=tts[3], op=AL.add)
            nc.vector.tensor_tensor(out=tts[0], in0=tts[0], in1=tts[2], op=AL.add)
            nc.sync.dma_start(out=o_ap[b * ROWS : (b + 1) * ROWS, :], in_=tts[0])

    _cap_sync_waits(nc)
    return nc


_CACHE = {}


def _get_nc():
    if "nc" not in _CACHE:
        _CACHE["nc"] = _build()
    return _CACHE["nc"]


def _pack_params(blocks):
    """Pack named (rows, cols) blocks into one (128, PARAM_K) f32 array."""
    A = np.zeros((128, PARAM_K), np.float32)
    for name, rows, cols in PARAM_LAYOUT:
        blk = blocks[name]
        assert blk.shape == (rows, cols), (name, blk.shape, rows, cols)
        off = PARAM_OFF[name]
        A[0:rows, off : off + cols] = blk
    return A


def _make_in_maps(q, k, v, modality_dropout,
                  q_w1, q_b1, q_w2, q_b2, q_g, q_beta,
                  k_w1, k_b1, k_w2, k_b2, k_g, k_beta):
    f = np.float32
    q = np.asarray(q, dtype=f)
    k = np.asarray(k, dtype=f)
    v = np.asarray(v, dtype=f)
    md = np.asarray(modality_dropout, dtype=f)
    const_blocks = {
        "qw1": np.asarray(q_w1, f).T,
        "qw2": np.asarray(q_w2, f).T,
        "kw1": np.asarray(k_w1, f).T,
        "kw2": np.asarray(k_w2, f).T,
        "qb1": np.asarray(q_b1, f).reshape(128, 1),
        "kb1": np.asarray(k_b1, f).reshape(128, 1),
        "qb2": np.tile(np.asarray(q_b2, f), (32, 1)),
        "qg": np.tile(np.asarray(q_g, f), (32, 1)),
        "qbe": np.tile(np.asarray(q_beta, f), (32, 1)),
        "kb2": np.tile(np.asarray(k_b2, f), (32, 1)),
        "kg": np.tile(np.asarray(k_g, f), (32, 1)),
        "kbe": np.tile(np.asarray(k_beta, f), (32, 1)),
        "eye": np.eye(32, dtype=f),
        "mask": np.repeat(np.eye(BPC, dtype=f), C, axis=1),
        "mask4": np.tile(np.eye(C, dtype=f), (BPC, 1)),
        "ones1": np.ones((1, 128), dtype=f),
        "eps": np.full((32, 1), LN_EPS, dtype=f),
    }
    in_maps = []
    for m in range(N_CORES):
        sl = slice(m * BPC, (m + 1) * BPC)
        blocks = dict(const_blocks)
        blocks["xq"] = q[sl, :, 0].T
        blocks["xk"] = np.transpose(k[sl, :, 0, :], (1, 0, 2)).reshape(
            DIM, C * BPC
        )
        blocks["md"] = md[sl]
        in_maps.append({
            "v": np.ascontiguousarray(v[sl]).reshape(BPC * ROWS, FW),
            "params": _pack_params(blocks),
        })
    return in_maps


def _run(in_maps, **kwargs):
    nc = _get_nc()
    return run_bass_kernel_spmd(nc, in_maps, core_ids=list(range(N_CORES)), **kwargs)


def _gather(res):
    out_full = np.empty((B, V_CH, IMG, IMG), np.float32)
    attn_full = np.empty((B, C), np.float32)
    for m in range(N_CORES):
        r = res.results[m]
        out_full[m * BPC : (m + 1) * BPC] = r["out"].reshape(BPC, V_CH, IMG, IMG)
        attn_full[m * BPC : (m + 1) * BPC] = r["attn"]
    attn_map = np.ascontiguousarray(
        np.broadcast_to(attn_full[:, :, None, None], (B, C, IMG, IMG))
    )
    return out_full, attn_map


def kernel(**inputs):
    in_maps = _make_in_maps(**inputs)
    res = _run(in_maps)
    return _gather(res)


def kernel_profiled(**inputs):
    """Like kernel(), but also returns BassKernelResults with trace info."""
    in_maps = _make_in_maps(**inputs)
    res = _run(in_maps, trace=True)
    return _gather(res), res


# revision 18
# speedup vs baseline: 1.1296x; 1.0845x over previous
"""Trainium2 Bass kernel for nn_AttentionModule (sparse_attention).

Computation (per batch b):
  qe = LN(MLP(q[b]))            (16,)
  ke = LN(MLP(k[b,:,0,:].T))    (4, 16)
  logits = qe @ ke.T * DIM^-0.5 - modality_dropout*1e5
  attn = softmax(logits / 10)   (4,)
  out[b, vc, p] = sum_c attn[c] * v[b, vc, p, c]
  attn_map = broadcast(attn)

Sharding: pure data parallel over batch across 8 NeuronCores (8 batches each).
MLP/LN params replicated. The heavy part (streaming 4 MB of v per batch) is
memory bound: per core 32 MB in + 8 MB out ~= 111 us at ~360 GB/s.

Device layout per batch: v[b] viewed flat as (128, 7840) f32 (partition-
contiguous DMA); channel c lives at free-dim stride 4. Weighted sum =
4 ScalarE multiplies (per-partition scale = attn weight broadcast via a
tiny TensorE outer-product) + 3 VectorE adds; result (128, 1960) DMAs out
contiguously. attn_map is a pure broadcast of the (64,4) attn tensor, so it
is materialized host-side from the device-computed attn.
"""

import numpy as np
from contextlib import ExitStack

import concourse.bass as bass
import concourse.tile as tile
from concourse import mybir
from concourse.bass_utils import run_bass_kernel_spmd

N_CORES = 8
B = 64
BPC = B // N_CORES  # batches per core
V_CH = 5
P = 50176
C = 4
DIM = 10
IMG = 224
ROWS = 128
FW = V_CH * P * C // ROWS   # 7840 f32 per partition per batch (with channels)
FO = V_CH * P // ROWS       # 1960 outputs per partition per batch
SCALE_T = float(DIM ** -0.5) / 10.0   # attn scale folded with temperature
MD_SCALE = 1e5 / 10.0
LN_EPS = 1e-5
F32 = mybir.dt.float32
AL = mybir.AluOpType
AF = mybir.ActivationFunctionType
AX = mybir.AxisListType


def _cap_sync_waits(nc, max_waits=1):
    """walrus in this container rejects >1 sync-wait per instruction ("Too
    many sync wait commands"). Spill excess waits onto same-engine nops
    inserted immediately before the instruction (engine streams execute bb
    instructions in order, so the waits still complete first)."""

    def _pop_by_name(name):
        for f2 in nc.m.functions:
            for b2 in f2.blocks:
                lst = b2.instructions
                for j in range(len(lst) - 1, -1, -1):
                    if lst[j].name == name:
                        return lst.pop(j)
        raise RuntimeError(f"spill nop {name} not found")

    for fn in nc.m.functions:
        for blk in fn.blocks:
            insts = blk.instructions
            i = 0
            while i < len(insts):
                ins = insts[i]
                si = ins.sync_info
                waits = list(si.on_wait) if si is not None and si.on_wait else []
                keep = 0 if type(ins).__name__ == "InstDrain" else max_waits
                if len(waits) > keep:
                    upds = (
                        list(si.on_update)
                        if si is not None and si.on_update
                        else []
                    )
                    spill = waits[keep:]
                    ins.sync_info = mybir.SyncInfo(
                        on_wait=waits[:keep], on_update=upds
                    )
                    for w in spill:
                        bi = nc.engines[ins.engine].nop(
                            nofuse=True, hint="wait_spill"
                        )
                        nop_inst = _pop_by_name(bi.ins.name)
                        nop_inst.sync_info = mybir.SyncInfo(
                            on_wait=[w], on_update=[]
                        )
                        insts.insert(i, nop_inst)
                        i += 1
                i += 1


def _layer_norm(nc, pool, x, g, beta, np_, name, eps_tile):
    """LayerNorm over the free dim (16) of x (np_, 16). Returns SBUF tile."""
    stats = pool.tile([np_, nc.vector.BN_STATS_DIM], F32, tag=name + "_st")
    nc.vector.bn_stats(out=stats, in_=x)
    mv = pool.tile([np_, nc.vector.BN_AGGR_DIM], F32, tag=name + "_mv")
    nc.vector.bn_aggr(out=mv, in_=stats)
    xc = pool.tile([np_, 16], F32, tag=name + "_xc")
    nc.vector.tensor_scalar(
        out=xc, in0=x, scalar1=mv[:, 0:1], scalar2=None, op0=AL.subtract
    )
    std = pool.tile([np_, 1], F32, tag=name + "_sd")
    nc.scalar.activation(
        out=std, in_=mv[:, 1:2], func=AF.Sqrt, bias=eps_tile[0:np_]
    )
    rstd = pool.tile([np_, 1], F32, tag=name + "_rs")
    nc.vector.reciprocal(out=rstd, in_=std)
    y = pool.tile([np_, 16], F32, tag=name + "_y")
    nc.vector.tensor_scalar(
        out=y, in0=xc, scalar1=rstd, scalar2=None, op0=AL.mult
    )
    nc.vector.tensor_tensor(out=y, in0=y, in1=g, op=AL.mult)
    nc.vector.tensor_tensor(out=y, in0=y, in1=beta, op=AL.add)
    return y


PARAM_LAYOUT = [
    # (name, rows, cols) packed side by side into one (128, PARAM_K) array
    ("xq", DIM, BPC),
    ("xk", DIM, C * BPC),
    ("md", BPC, C),
    ("qw1", DIM, 128),
    ("qw2", 128, 16),
    ("kw1", DIM, 128),
    ("kw2", 128, 16),
    ("qb1", 128, 1),
    ("kb1", 128, 1),
    ("qb2", 32, 16),
    ("qg", 32, 16),
    ("qbe", 32, 16),
    ("kb2", 32, 16),
    ("kg", 32, 16),
    ("kbe", 32, 16),
    ("eye", 32, 32),
    ("mask", BPC, C * BPC),
    ("mask4", C * BPC, C),
    ("ones1", 1, 128),
    ("eps", 32, 1),
]
PARAM_OFF = {}
_off = 0
for _n, _r, _c in PARAM_LAYOUT:
    PARAM_OFF[_n] = _off
    _off += _c
PARAM_K = _off


def _build():
    nc = bass.Bass("TRN2", target_bir_lowering=False, debug=False)

    v_d = nc.dram_tensor("v", [BPC * ROWS, FW], F32, kind="ExternalInput")
    params_d = nc.dram_tensor("params", [128, PARAM_K], F32, kind="ExternalInput")

    out_d = nc.dram_tensor("out", [BPC * ROWS, FO], F32, kind="ExternalOutput")
    attn_d = nc.dram_tensor("attn", [BPC, C], F32, kind="ExternalOutput")

    with tile.TileContext(nc) as tc, ExitStack() as ctx:
        singles = ctx.enter_context(tc.tile_pool(name="singles", bufs=1))
        small = ctx.enter_context(tc.tile_pool(name="small", bufs=1))
        psum = ctx.enter_context(tc.tile_pool(name="psum", bufs=1, space="PSUM"))
        vpool = ctx.enter_context(tc.tile_pool(name="vpool", bufs=4))
        tpool = ctx.enter_context(tc.tile_pool(name="tpool", bufs=2))

        params = singles.tile([128, PARAM_K], F32, tag="params")
        nc.sync.dma_start(out=params, in_=params_d.ap())

        def pslice(name, rows, cols):
            off = PARAM_OFF[name]
            return params[0:rows, off : off + cols]

        views = {n: pslice(n, r, c) for n, r, c in PARAM_LAYOUT}
        xq = views["xq"]
        xk = views["xk"]
        mdt = views["md"]
        qw1 = views["qw1"]
        qw2 = views["qw2"]
        kw1 = views["kw1"]
        kw2 = views["kw2"]
        qb1 = views["qb1"]
        kb1 = views["kb1"]
        qb2 = views["qb2"]
        qg = views["qg"]
        qbe = views["qbe"]
        kb2 = views["kb2"]
        kg = views["kg"]
        kbe = views["kbe"]
        eye = views["eye"]
        mask = views["mask"]
        mask4 = views["mask4"]
        ones1 = views["ones1"]
        eps_t = views["eps"]

        # ---- q path MLP + LN -> qe (BPC, 16)
        h1q_ps = psum.tile([128, BPC], F32, tag="psA")
        nc.tensor.matmul(out=h1q_ps, lhsT=qw1, rhs=xq)
        # LeakyReLU(0.1) done manually: the ACT Lrelu table bakes alpha=0.01.
        h1q = small.tile([128, BPC], F32, tag="h1q_sb")
        h1qt = small.tile([128, BPC], F32, tag="h1q_t")
        nc.vector.tensor_scalar(
            out=h1q, in0=h1q_ps, scalar1=qb1, scalar2=None, op0=AL.add
        )
        nc.vector.tensor_scalar(
            out=h1qt, in0=h1q, scalar1=0.1, scalar2=None, op0=AL.mult
        )
        nc.vector.tensor_tensor(out=h1q, in0=h1q, in1=h1qt, op=AL.max)
        h2q_ps = psum.tile([BPC, 16], F32, tag="psB")
        nc.tensor.matmul(out=h2q_ps, lhsT=h1q, rhs=qw2)
        hq = small.tile([BPC, 16], F32, tag="hq")
        nc.vector.tensor_tensor(out=hq, in0=h2q_ps, in1=qb2[0:BPC], op=AL.add)
        qe = _layer_norm(nc, small, hq, qg[0:BPC], qbe[0:BPC], BPC, "qln", eps_t)

        # ---- k path MLP + LN -> ke (C*BPC, 16), row j = 4*b + c
        h1k_ps = psum.tile([128, C * BPC], F32, tag="psA")
        nc.tensor.matmul(out=h1k_ps, lhsT=kw1, rhs=xk)
        h1k = small.tile([128, C * BPC], F32, tag="h1k_sb")
        h1kt = small.tile([128, C * BPC], F32, tag="h1k_t")
        nc.vector.tensor_scalar(
            out=h1k, in0=h1k_ps, scalar1=kb1, scalar2=None, op0=AL.add
        )
        nc.vector.tensor_scalar(
            out=h1kt, in0=h1k, scalar1=0.1, scalar2=None, op0=AL.mult
        )
        nc.vector.tensor_tensor(out=h1k, in0=h1k, in1=h1kt, op=AL.max)
        h2k_ps = psum.tile([C * BPC, 16], F32, tag="psB")
        nc.tensor.matmul(out=h2k_ps, lhsT=h1k, rhs=kw2)
        hk = small.tile([C * BPC, 16], F32, tag="hk")
        nc.vector.tensor_tensor(out=hk, in0=h2k_ps, in1=kb2, op=AL.add)
        ke = _layer_norm(nc, small, hk, kg, kbe, C * BPC, "kln", eps_t)

        # ---- logits: L2[b', 4b+c] = qe[b'] . ke[4b+c]; keep diagonal b'=b
        qeT_ps = psum.tile([16, BPC], F32, tag="psC")
        nc.tensor.transpose(out=qeT_ps, in_=qe, identity=eye[0:BPC, 0:BPC])
        qeT = small.tile([16, BPC], F32, tag="qeT_sb")
        nc.vector.tensor_copy(out=qeT, in_=qeT_ps)
        keT_ps = psum.tile([16, C * BPC], F32, tag="psD")
        nc.tensor.transpose(out=keT_ps, in_=ke, identity=eye)
        keT = small.tile([16, C * BPC], F32, tag="keT_sb")
        nc.vector.tensor_copy(out=keT, in_=keT_ps)
        l2_ps = psum.tile([BPC, C * BPC], F32, tag="psB")
        nc.tensor.matmul(out=l2_ps, lhsT=qeT, rhs=keT)
        zm = small.tile([BPC, C * BPC], F32, tag="zm")
        nc.vector.tensor_tensor(out=zm, in0=l2_ps, in1=mask, op=AL.mult)
        z = small.tile([BPC, C], F32, tag="z")
        nc.vector.tensor_reduce(
            out=z,
            in_=zm.rearrange("p (b2 c) -> p c b2", c=C),
            axis=AX.X,
            op=AL.add,
        )

        # ---- masked softmax over the 4 modalities
        nc.vector.tensor_scalar(
            out=z, in0=z, scalar1=SCALE_T, scalar2=None, op0=AL.mult
        )
        mds = small.tile([BPC, C], F32, tag="mds")
        nc.vector.tensor_scalar(
            out=mds, in0=mdt, scalar1=MD_SCALE, scalar2=None, op0=AL.mult
        )
        nc.vector.tensor_tensor(out=z, in0=z, in1=mds, op=AL.subtract)
        rmax = small.tile([BPC, 1], F32, tag="rmax")
        nc.vector.tensor_reduce(out=rmax, in_=z, axis=AX.X, op=AL.max)
        nmax = small.tile([BPC, 1], F32, tag="nmax")
        nc.vector.tensor_scalar(
            out=nmax, in0=rmax, scalar1=-1.0, scalar2=None, op0=AL.mult
        )
        e = small.tile([BPC, C], F32, tag="e")
        nc.scalar.activation(out=e, in_=z, func=AF.Exp, bias=nmax, scale=1.0)
        ssum = small.tile([BPC, 1], F32, tag="ssum")
        nc.vector.tensor_reduce(out=ssum, in_=e, axis=AX.X, op=AL.add)
        rs = small.tile([BPC, 1], F32, tag="rs")
        nc.vector.reciprocal(out=rs, in_=ssum)
        attn_sb = small.tile([BPC, C], F32, tag="attn_sb")
        nc.vector.tensor_scalar(
            out=attn_sb, in0=e, scalar1=rs, scalar2=None, op0=AL.mult
        )
        nc.sync.dma_start(out=attn_d.ap(), in_=attn_sb)

        # ---- broadcast attn to all 128 partitions: wall[p, 4b+c] = attn[b, c]
        # 1) scatter rows: o32[4b+c', c] = attn[b, c] (mask == repeat(eye(8),4))
        o32_ps = psum.tile([C * BPC, C], F32, tag="psB")
        nc.tensor.matmul(out=o32_ps, lhsT=mask, rhs=attn_sb)
        # 2) keep diagonal c' == j%4 and reduce -> attn32[4b+c] = attn[b, c]
        o32 = small.tile([C * BPC, C], F32, tag="o32_sb")
        nc.vector.tensor_tensor(out=o32, in0=o32_ps, in1=mask4, op=AL.mult)
        attn32 = small.tile([C * BPC, 1], F32, tag="attn32")
        nc.vector.tensor_reduce(out=attn32, in_=o32, axis=AX.X, op=AL.add)
        # 3) flatten to one partition, then outer-product with a ones column
        arow_ps = psum.tile([1, C * BPC], F32, tag="psC")
        nc.tensor.transpose(out=arow_ps, in_=attn32, identity=eye)
        arow = small.tile([1, C * BPC], F32, tag="arow_sb")
        nc.vector.tensor_copy(out=arow, in_=arow_ps)
        wps = psum.tile([128, C * BPC], F32, tag="psD")
        nc.tensor.matmul(out=wps, lhsT=ones1, rhs=arow)
        wall = singles.tile([128, C * BPC], F32, tag="wall_sb")
        nc.vector.tensor_copy(out=wall, in_=wps)

        # ---- big weighted sum over v, one batch at a time
        v_ap = v_d.ap()
        o_ap = out_d.ap()
        for b in range(BPC):
            T = vpool.tile([ROWS, FW], F32, tag="T")
            nc.sync.dma_start(out=T, in_=v_ap[b * ROWS : (b + 1) * ROWS, :])
            Tv = T.rearrange("p (f c) -> p c f", c=C)  # channel slice = stride 4
            tts = []
            for cc in range(C):
                tcc = tpool.tile([ROWS, FO], F32, tag=f"t{cc}")
                w_ap = wall[:, C * b + cc : C * b + cc + 1]
                ch = Tv[:, cc : cc + 1, :].squeeze(1)
                if cc % 2 == 0:
                    nc.scalar.mul(out=tcc, in_=ch, mul=w_ap)
                else:
                    nc.vector.tensor_scalar(
                        out=tcc, in0=ch, scalar1=w_ap, scalar2=None, op0=AL.mult
                    )
                tts.append(tcc)
            nc.vector.tensor_tensor(out=tts[0], in0=tts[0], in1=tts[1], op=AL.add)
            nc.vector.tensor_tensor(out=tts[2], in0=tts[2], in1=tts[3], op=AL.add)
            nc.vector.tensor_tensor(out=tts[0], in0=tts[0], in1=tts[2], op=AL.add)
            # out-DMA via the POOL SWDGE queue stream: keeps the SP HWDGE
            # ring (FIFO) exclusively feeding 4MB v-input transfers.
            nc.gpsimd.dma_start(out=o_ap[b * ROWS : (b + 1) * ROWS, :], in_=tts[0])

    _cap_sync_waits(nc)
    return nc


_CACHE = {}


def _get_nc():
    if "nc" not in _CACHE:
        _CACHE["nc"] = _build()
    return _CACHE["nc"]


def _pack_params(blocks):
    """Pack named (rows, cols) blocks into one (128, PARAM_K) f32 array."""
    A = np.zeros((128, PARAM_K), np.float32)
    for name, rows, cols in PARAM_LAYOUT:
        blk = blocks[name]
        assert blk.shape == (rows, cols), (name, blk.shape, rows, cols)
        off = PARAM_OFF[name]
        A[0:rows, off : off + cols] = blk
    return A


def _make_in_maps(q, k, v, modality_dropout,
                  q_w1, q_b1, q_w2, q_b2, q_g, q_beta,
                  k_w1, k_b1, k_w2, k_b2, k_g, k_beta):
    f = np.float32
    q = np.asarray(q, dtype=f)
    k = np.asarray(k, dtype=f)
    v = np.asarray(v, dtype=f)
    md = np.asarray(modality_dropout, dtype=f)
    const_blocks = {
        "qw1": np.asarray(q_w1, f).T,
        "qw2": np.asarray(q_w2, f).T,
        "kw1": np.asarray(k_w1, f).T,
        "kw2": np.asarray(k_w2, f).T,
        "qb1": np.asarray(q_b1, f).reshape(128, 1),
        "kb1": np.asarray(k_b1, f).reshape(128, 1),
        "qb2": np.tile(np.asarray(q_b2, f), (32, 1)),
        "qg": np.tile(np.asarray(q_g, f), (32, 1)),
        "qbe": np.tile(np.asarray(q_beta, f), (32, 1)),
        "kb2": np.tile(np.asarray(k_b2, f), (32, 1)),
        "kg": np.tile(np.asarray(k_g, f), (32, 1)),
        "kbe": np.tile(np.asarray(k_beta, f), (32, 1)),
        "eye": np.eye(32, dtype=f),
        "mask": np.repeat(np.eye(BPC, dtype=f), C, axis=1),
        "mask4": np.tile(np.eye(C, dtype=f), (BPC, 1)),
        "ones1": np.ones((1, 128), dtype=f),
        "eps": np.full((32, 1), LN_EPS, dtype=f),
    }
    in_maps = []
    for m in range(N_CORES):
        sl = slice(m * BPC, (m + 1) * BPC)
        blocks = dict(const_blocks)
        blocks["xq"] = q[sl, :, 0].T
        blocks["xk"] = np.transpose(k[sl, :, 0, :], (1, 0, 2)).reshape(
            DIM, C * BPC
        )
        blocks["md"] = md[sl]
        in_maps.append({
            "v": np.ascontiguousarray(v[sl]).reshape(BPC * ROWS, FW),
            "params": _pack_params(blocks),
        })
    return in_maps


def _run(in_maps, **kwargs):
    nc = _get_nc()
    return run_bass_kernel_spmd(nc, in_maps, core_ids=list(range(N_CORES)), **kwargs)


def _gather(res):
    out_full = np.empty((B, V_CH, IMG, IMG), np.float32)
    attn_full = np.empty((B, C), np.float32)
    for m in range(N_CORES):
        r = res.results[m]
        out_full[m * BPC : (m + 1) * BPC] = r["out"].reshape(BPC, V_CH, IMG, IMG)
        attn_full[m * BPC : (m + 1) * BPC] = r["attn"]
    attn_map = np.ascontiguousarray(
        np.broadcast_to(attn_full[:, :, None, None], (B, C, IMG, IMG))
    )
    return out_full, attn_map


def kernel(**inputs):
    in_maps = _make_in_maps(**inputs)
    res = _run(in_maps)
    return _gather(res)


def kernel_profiled(**inputs):
    """Like kernel(), but also returns BassKernelResults with trace info."""
    in_maps = _make_in_maps(**inputs)
    res = _run(in_maps, trace=True)
    return _gather(res), res
